# revision 30
# baseline (speedup 1.0000x reference)
"""Trainium2 Bass kernel for the BDH-style sparse-attention model.

Model (per reference): L=6 layers over T=1024 tokens, D=256, H=4 heads,
N=32768 neurons (NH=8192 per head), strict-causal unnormalized linear
attention with RoPE over the neuron dim, gated wide projection, encoder
contraction with residual layernorms, final vocab readout.

Sharding (8 NeuronCores): tensor-parallel over (head, neuron): core c owns
head h=c//2 and half of that head's neurons (4096), chosen as a contiguous
slice of the rope *pair* space so rotary stays core-local:
  pair p=c%2 owns head-cols [p*2048,(p+1)*2048) and [4096+p*2048, ...+2048).

Per layer (phases, emitted software-pipelined so collectives hide):
  A(tci): x = relu(v @ Wx) [PE, Wx persistent in SBUF]; xr = rope(x) [DVE,
          trig streamed]; G = xr xr^T strict-upper [PE]; a-partial = S^T v
          [PE] -> pairwise AllReduce
  B(tci): y = relu(ln(a) @ Wy) * x [PE+DVE]; e-partial[t,d] = y^T enc [PE,
          lands in residual layout] -> 8-core AllReduce (bf16)
  C(tci): v = ln(v + ln(e)) in place; vb -> vt transposes [PE]
Emission: A(l,0) C(l-1,1) A(l,1) B(l,0) B(l,1) C(l,0) ... so each collective
has ~50us of independent PE work behind it.

Output (bf16, host-cast to f32): v @ readout, first half emitted right
after C(L-1,0).
"""

import numpy as np
import ml_dtypes

import concourse.bass as bass
import concourse.mybir as mybir
import concourse.tile as tile
from concourse import bacc
from concourse.bass_utils import run_bass_kernel_spmd

AF = mybir.ActivationFunctionType
ALU = mybir.AluOpType
F32 = mybir.dt.float32
BF16 = mybir.dt.bfloat16

NCORES = 8
D = 256
H = 4
L = 6
N = 32768
NH = N // H          # 8192
NLOC = NH // 2       # 4096 per-core neurons
HALF = NLOC // 2     # 2048 rope pairs per core
T = 1024
VOCAB = 256
ROPE_BASE = 10000.0
NCH = NLOC // 128    # 32 chunks of 128 neurons
NPAIR = NCH // 2     # 16 rope chunk pairs
TCN = 2              # t-chunks
TCW = T // TCN       # 512
TT = T // 128        # 8 global t-tiles

REPLICA_PAIRS = [[0, 1], [2, 3], [4, 5], [6, 7]]
REPLICA_ALL = [list(range(NCORES))]


def build(nlayers: int = L, collectives: bool = True):
    nc = bacc.Bacc(
        "TRN2", target_bir_lowering=False, debug=False,
        enable_asserts=False, num_devices=NCORES,
    )

    # ---- DRAM I/O ----
    wx_d = nc.dram_tensor("wx", [128, NCH, 2, 128], BF16, kind="ExternalInput")
    wy_d = nc.dram_tensor("wy", [128, NCH, 2, 128], BF16, kind="ExternalInput")
    enc_d = nc.dram_tensor("enc", [NCH, 128, D], BF16, kind="ExternalInput")
    # cos/sin fused: [pair, tci, 128, {cos,sin}, TCW] so a single DMA per
    # (pair, tci) lands straight into a [128, 2, TCW] SBUF tile.
    trg_d = nc.dram_tensor("trg", [NPAIR, TCN, 128, 2, TCW], BF16,
                           kind="ExternalInput")
    ro_d = nc.dram_tensor("ro", [128, 2, VOCAB], BF16, kind="ExternalInput")
    v0b_d = nc.dram_tensor("v0b", [128, TT, D], BF16, kind="ExternalInput")
    v0t_d = nc.dram_tensor("v0t", [128, 2, T], BF16, kind="ExternalInput")
    mask_d = nc.dram_tensor("maskd", [128, 128], BF16, kind="ExternalInput")
    ident_d = nc.dram_tensor("ident", [128, 128], BF16, kind="ExternalInput")
    out_d = nc.dram_tensor("out", [T, VOCAB], BF16, kind="ExternalOutput")

    trg_ap, enc_ap = trg_d.ap(), enc_d.ap()

    with tile.TileContext(nc) as tc:
        with (
            tc.tile_pool(name="pers", bufs=1) as pers,
            tc.tile_pool(name="chbf", bufs=2) as chbf,
            tc.tile_pool(name="trig", bufs=2) as trig,
            tc.tile_pool(name="encp", bufs=4) as encp,
            tc.tile_pool(name="s32", bufs=1) as s32,
            tc.tile_pool(name="sbf", bufs=2) as sbf,
            tc.tile_pool(name="stat", bufs=2) as statp,
            tc.tile_pool(name="pxy", bufs=6, space="PSUM") as pxy_pool,
            tc.tile_pool(name="pap", bufs=2, space="PSUM") as pap_pool,
            tc.tile_pool(name="dram", bufs=2, space="DRAM") as dram,
        ):
            # ---- persistent SBUF ----
            ro = pers.tile([128, 2, VOCAB], BF16, tag="ro")
            maskd = pers.tile([128, 128], BF16, tag="maskd")
            ident = pers.tile([128, 128], BF16, tag="ident")
            wx = pers.tile([128, NCH, 2, 128], BF16, tag="wx")
            wy = pers.tile([128, NCH, 2, 128], BF16, tag="wy")
            xT = pers.tile([128, NCH, T], BF16, tag="xT")
            xrT = pers.tile([128, NCH, T], BF16, tag="xrT")
            # S packed by s-tile row: row st holds t in [st*128, T) at offset
            # soff[st]; total 4608 cols (vs 8192 dense).
            soff = [0]
            for st in range(1, TT + 1):
                soff.append(soff[-1] + T - 128 * (st - 1))
            S = pers.tile([128, soff[TT]], BF16, tag="S")

            def S_ap(st, t0, w):
                base = soff[st] + t0 - st * 128
                return S[:, base:base + w]

            vb = pers.tile([128, TT, D], BF16, tag="vb")
            vt = pers.tile([128, 2, T], BF16, tag="vt")
            eps = pers.tile([128, 1], F32, tag="eps")
            nc.gpsimd.memset(eps[:], 1e-5)

            # wx/v0t first: layer-0 X depends on them
            nc.sync.dma_start(wx[:], wx_d[:])
            nc.sync.dma_start(vt[:], v0t_d[:])
            nc.sync.dma_start(vb[:], v0b_d[:])
            nc.sync.dma_start(wy[:], wy_d[:])
            nc.sync.dma_start(maskd[:], mask_d[:])
            nc.sync.dma_start(ident[:], ident_d[:])
            nc.sync.dma_start(ro[:], ro_d[:])

            def ln_stats(src, nt, tag):
                """src [128, nt, D] -> (rstd, -mean*rstd) each [128, nt]."""
                bns = statp.tile([128, nt, 6], F32, tag=f"bns{tag}", bufs=1)
                agg = statp.tile([128, nt, 2], F32, tag=f"agg{tag}", bufs=1)
                for i in range(nt):
                    nc.vector.bn_stats(bns[:, i, :], src[:, i, :])
                    nc.vector.bn_aggr(agg[:, i, :], bns[:, i, :])
                std = statp.tile([128, nt], F32, tag=f"std{tag}", bufs=1)
                rstd = statp.tile([128, nt], F32, tag=f"rstd{tag}", bufs=1)
                nmr = statp.tile([128, nt], F32, tag=f"nmr{tag}", bufs=1)
                nc.scalar.activation(std[:], agg[:, :, 1], AF.Sqrt, bias=eps[:])
                nc.vector.reciprocal(rstd[:], std[:])
                nc.vector.scalar_tensor_tensor(
                    nmr[:], agg[:, :, 0], -1.0, rstd[:], ALU.mult, ALU.mult)
                return rstd, nmr

            def ln_apply(dst_ap_fn, src, nt, rstd, nmr):
                for i in range(nt):
                    nc.scalar.activation(
                        dst_ap_fn(i), src[:, i, :], AF.Identity,
                        bias=nmr[:, i:i + 1], scale=rstd[:, i:i + 1])

            def transpose_block(src_ap, dst_ap, name):
                ps = pap_pool.tile([128, 128], BF16, tag="pap", name=name)
                nc.tensor.transpose(ps[:], src_ap, ident[:])
                nc.any.tensor_copy(dst_ap, ps[:])

            a_red = {}
            e_red = {}

            def phaseA(l, tci):
                """X + rope; G; a-partial; start AllReduce(a)."""
                t0c = tci * TCW
                sl = slice(t0c, t0c + TCW)
                for c in range(NPAIR):
                    for cc in (c, NPAIR + c):
                        ps = pxy_pool.tile([128, TCW], F32, tag="pxy",
                                           name=f"px_{cc}_{tci}")
                        for dc in range(2):
                            nc.tensor.matmul(
                                ps[:], wx[:, cc, dc, :],
                                vt[:, dc, sl],
                                start=(dc == 0), stop=(dc == 1),
                            )
                        nc.scalar.activation(xT[:, cc, sl], ps[:], AF.Relu)
                    tg = trig.tile([128, 2, TCW], BF16, tag="trg",
                                   bufs=4, name=f"tg_{c}")
                    nc.sync.dma_start(tg[:], trg_ap[c, tci])
                    co, si = tg[:, 0, :], tg[:, 1, :]
                    p1 = chbf.tile([128, TCW], BF16, tag="p1", bufs=1,
                                   name=f"p1_{c}")
                    p2 = chbf.tile([128, TCW], BF16, tag="p2", bufs=1,
                                   name=f"p2_{c}")
                    # xr1 = x1*cos - x2*sin
                    nc.vector.tensor_tensor(p1[:], xT[:, c, sl], co, ALU.mult)
                    nc.vector.tensor_tensor(
                        p2[:], xT[:, NPAIR + c, sl], si, ALU.mult)
                    nc.vector.tensor_tensor(
                        xrT[:, c, sl], p1[:], p2[:], ALU.subtract)
                    # xr2 = x2*cos + x1*sin
                    nc.vector.tensor_tensor(
                        p1[:], xT[:, NPAIR + c, sl], co, ALU.mult)
                    nc.vector.tensor_tensor(p2[:], xT[:, c, sl], si, ALU.mult)
                    nc.vector.tensor_tensor(
                        xrT[:, NPAIR + c, sl], p1[:], p2[:], ALU.add)

                # G blocks (strict upper in (s, t)); chunk-major in groups of
                # two PSUM banks so accumulation rides the rope stream.
                sts = list(range(4 * tci + 4))
                for g0 in range(0, len(sts), 3):
                    grp = sts[g0:g0 + 3]
                    pgs = {}
                    geom = {}
                    for st in grp:
                        tg0 = max(st * 128, t0c)
                        nw = t0c + TCW - tg0
                        geom[st] = (tg0, nw)
                        pgs[st] = pxy_pool.tile(
                            [128, TCW], F32, tag="pxy", name=f"pg_{st}")
                    for cc in range(NCH):
                        for st in grp:
                            tg0, nw = geom[st]
                            nc.tensor.matmul(
                                pgs[st][:, :nw],
                                xrT[:, cc, st * 128:(st + 1) * 128],
                                xrT[:, cc, tg0:tg0 + nw],
                                start=(cc == 0), stop=(cc == NCH - 1),
                            )
                    for st in grp:
                        tg0, nw = geom[st]
                        pg = pgs[st]
                        if tg0 == st * 128:
                            nc.vector.tensor_tensor(
                                S_ap(st, tg0, 128), pg[:, 0:128], maskd[:],
                                ALU.mult)
                            if nw > 128:
                                nc.any.tensor_copy(
                                    S_ap(st, tg0 + 128, nw - 128),
                                    pg[:, 128:nw])
                        else:
                            nc.any.tensor_copy(
                                S_ap(st, tg0, nw), pg[:, :nw])

            def phaseAa(l, tci):
                """a-partial from S and v; start AllReduce(a)."""
                a_loc = sbf.tile([128, 4, D], BF16, tag="a_loc", bufs=1,
                                 name=f"a_loc_{l}_{tci}")
                for i in range(4):
                    gt = 4 * tci + i
                    pa = pap_pool.tile([128, D], F32, tag="pap",
                                       name=f"pa_{gt}")
                    for st in range(gt + 1):
                        nc.tensor.matmul(
                            pa[:], S_ap(st, gt * 128, 128),
                            vb[:, st, :],
                            start=(st == 0), stop=(st == gt),
                        )
                    nc.any.tensor_copy(a_loc[:, i, :], pa[:])

                if collectives:
                    ain = dram.tile([128, 4, D], BF16, tag="ain",
                                    name=f"ain_{l}_{tci}")
                    aout = dram.tile([128, 4, D], BF16, tag="aout",
                                     name=f"aout_{l}_{tci}")
                    nc.sync.dma_start(ain[:], a_loc[:])
                    if collectives == "dma":
                        nc.sync.dma_start(aout[:], ain[:])
                    else:
                        nc.gpsimd.collective_compute(
                            "AllReduce", ALU.add, replica_groups=REPLICA_PAIRS,
                            ins=[ain.opt()], outs=[aout.opt()])
                    ar = sbf.tile([128, 4, D], BF16, tag="a_red", bufs=2,
                                  name=f"a_red_{l}_{tci}")
                    nc.sync.dma_start(ar[:], aout[:])
                    a_red[tci] = ar
                else:
                    a_red[tci] = a_loc

            def phaseB(l, tci, filler=None):
                """ln(a); Y + gate; e-partial in [t,d]; start AllReduce(e)."""
                t0c = tci * TCW
                sl = slice(t0c, t0c + TCW)
                ar = a_red[tci]
                rstd, nmr = ln_stats(ar, 4, "a")
                lnA = sbf.tile([128, 4, D], BF16,
                               tag="a_loc" if collectives else "lnA",
                               bufs=1, name=f"lnA_{l}_{tci}")
                ln_apply(lambda i: lnA[:, i, :], ar, 4, rstd, nmr)
                lat = sbf.tile([128, 2, TCW], BF16, tag="lnAT", bufs=1,
                               name=f"lnAT_{l}_{tci}")
                for i in range(4):
                    for dc in range(2):
                        transpose_block(
                            lnA[:, i, dc * 128:(dc + 1) * 128],
                            lat[:, dc, i * 128:(i + 1) * 128],
                            f"ptA_{i}_{dc}")
                if filler is not None:
                    # PE filler while the post-collective ln chain resolves
                    filler()

                # one full PSUM bank per t-tile accumulator (a bank can hold
                # only one pending accumulation group)
                pets = [
                    pxy_pool.tile([128, TCW], F32, tag="pxy",
                                  name=f"pet_{tci}_{ti}")[:, :D]
                    for ti in range(4)
                ]
                for c in range(NCH):
                    py = pxy_pool.tile([128, TCW], F32, tag="pxy",
                                       name=f"py_{c}_{tci}")
                    for dc in range(2):
                        nc.tensor.matmul(
                            py[:], wy[:, c, dc, :], lat[:, dc, :],
                            start=(dc == 0), stop=(dc == 1),
                        )
                    yc = chbf.tile([128, TCW], BF16, tag="yc", name=f"yc_{c}")
                    nc.vector.scalar_tensor_tensor(
                        yc[:], py[:], 0.0, xT[:, c, sl], ALU.max, ALU.mult)
                    ec = encp.tile([128, D], BF16, tag="enc", name=f"ec_{c}")
                    nc.scalar.dma_start(ec[:], enc_ap[c, :, :])
                    # e[t, d] += yc[:, ti]^T @ enc: lands already in the vb
                    # layout -> no transposes on the collective path
                    for ti in range(4):
                        nc.tensor.matmul(
                            pets[ti], yc[:, ti * 128:(ti + 1) * 128],
                            ec[:],
                            start=(c == 0), stop=(c == NCH - 1),
                        )
                eT = s32.tile([128, 4, D], BF16, tag="eT", bufs=2,
                              name=f"eT_{l}_{tci}")
                for ti in range(4):
                    nc.any.tensor_copy(eT[:, ti, :], pets[ti])
                if collectives:
                    ein = dram.tile([128, 4, D], BF16, tag="ein",
                                    name=f"ein_{l}_{tci}")
                    eout = dram.tile([128, 4, D], BF16, tag="eout",
                                     name=f"eout_{l}_{tci}")
                    nc.sync.dma_start(ein[:], eT[:])
                    if collectives == "dma":
                        nc.sync.dma_start(eout[:], ein[:])
                    else:
                        nc.gpsimd.collective_compute(
                            "AllReduce", ALU.add, replica_groups=REPLICA_ALL,
                            ins=[ein.opt()], outs=[eout.opt()])
                    ert = s32.tile([128, 4, D], BF16, tag="eT", bufs=2,
                                   name=f"ert_{l}_{tci}")
                    nc.sync.dma_start(ert[:], eout[:])
                    e_red[tci] = ert
                else:
                    e_red[tci] = eT

            def phaseC(l, tci):
                """v = ln(v + ln(e)) in place; refresh vb and vt."""
                er = e_red[tci]
                rstd, nmr = ln_stats(er, 4, "e")
                ln_apply(lambda i: er[:, i, :], er, 4, rstd, nmr)
                nc.vector.tensor_tensor(
                    er[:], vb[:, 4 * tci:4 * tci + 4, :], er[:], ALU.add)
                rstd2, nmr2 = ln_stats(er, 4, "v")
                for i in range(4):
                    gt = 4 * tci + i
                    nc.scalar.activation(
                        vb[:, gt, :], er[:, i, :], AF.Identity,
                        bias=nmr2[:, i:i + 1], scale=rstd2[:, i:i + 1])
                    for dc in range(2):
                        transpose_block(
                            vb[:, gt, dc * 128:(dc + 1) * 128],
                            vt[:, dc, gt * 128:(gt + 1) * 128],
                            f"ptV_{gt}_{dc}")

            def readout(half):
                ob = s32.tile([128, 4, D], BF16, tag="eT", bufs=2,
                              name=f"ob_{half}")
                for i in range(4):
                    gt = 4 * half + i
                    ps = pap_pool.tile([128, VOCAB], F32, tag="pap",
                                       name=f"pro_{gt}")
                    for dc in range(2):
                        nc.tensor.matmul(
                            ps[:], vt[:, dc, gt * 128:(gt + 1) * 128],
                            ro[:, dc, :],
                            start=(dc == 0), stop=(dc == 1),
                        )
                    nc.any.tensor_copy(ob[:, i, :], ps[:])
                    nc.sync.dma_start(
                        out_d[gt * 128:(gt + 1) * 128, :], ob[:, i, :])

            # ---- software-pipelined emission ----
            for l in range(nlayers):
                phaseA(l, 0)
                phaseAa(l, 0)
                if l > 0:
                    phaseC(l - 1, 1)
                phaseA(l, 1)
                phaseB(l, 0, filler=lambda l=l: phaseAa(l, 1))
                phaseB(l, 1)
                phaseC(l, 0)
                if l == nlayers - 1:
                    readout(0)
            phaseC(nlayers - 1, 1)
            readout(1)

    nc.compile()
    return nc


def prep_inputs(inputs):
    """Full inputs -> per-core in_maps (host-side shard + precompute)."""
    bf = ml_dtypes.bfloat16
    idx = np.asarray(inputs["idx"], dtype=np.int32)
    wte = np.asarray(inputs["wte"], dtype=np.float32)
    enc = np.asarray(inputs["encoder"], dtype=np.float32)
    dx = np.asarray(inputs["decoder_x"], dtype=np.float32)
    dy = np.asarray(inputs["decoder_y"], dtype=np.float32)
    ro = np.asarray(inputs["readout"], dtype=np.float32)

    # embedding + initial layernorm (host)
    v0 = wte[idx[0]]
    m = v0.mean(-1, keepdims=True)
    va = v0.var(-1, keepdims=True)
    v0 = ((v0 - m) / np.sqrt(va + 1e-5)).astype(np.float32)  # [T, D]
    v0b = np.ascontiguousarray(
        v0.reshape(TT, 128, D).transpose(1, 0, 2)).astype(bf)
    v0t = np.ascontiguousarray(
        v0.T.reshape(2, 128, T).transpose(1, 0, 2)).astype(bf)

    half_g = NH // 2
    inv = 1.0 / (ROPE_BASE ** (np.arange(half_g, dtype=np.float32) / half_g))
    tarr = np.arange(T, dtype=np.float32)

    mask = np.triu(np.ones((128, 128), np.float32), k=1).astype(bf)
    ident = np.eye(128, dtype=np.float32).astype(bf)
    ro_arr = np.ascontiguousarray(
        ro.reshape(2, 128, VOCAB).transpose(1, 0, 2)).astype(bf)

    in_maps = []
    for c in range(NCORES):
        h, p = c // 2, c % 2
        j0, j1 = p * HALF, (p + 1) * HALF
        cols = np.r_[j0:j1, half_g + j0:half_g + j1]
        wx_c = dx[h][:, cols]   # [256, 4096]
        wy_c = dy[h][:, cols]
        enc_c = enc[h * NH:(h + 1) * NH][cols]  # [4096, 256]

        # [256, 4096] -> [128, NCH, 2, 128]: [d, n] with d=128*dc+p, n=128*ch+i
        wx_arr = np.ascontiguousarray(
            wx_c.reshape(2, 128, NCH, 128).transpose(1, 2, 0, 3)).astype(bf)
        wy_arr = np.ascontiguousarray(
            wy_c.reshape(2, 128, NCH, 128).transpose(1, 2, 0, 3)).astype(bf)
        enc_arr = np.ascontiguousarray(enc_c.reshape(NCH, 128, D)).astype(bf)

        ang = tarr[:, None] * inv[None, j0:j1]      # [T, 2048]
        cos = np.cos(ang).T.astype(np.float32)      # [2048, T]
        sin = np.sin(ang).T.astype(np.float32)
        cs_arr = cos.reshape(NPAIR, 128, TCN, TCW).transpose(0, 2, 1, 3)
        sn_arr = sin.reshape(NPAIR, 128, TCN, TCW).transpose(0, 2, 1, 3)
        # [NPAIR, TCN, 128, 2, TCW] with cos at [..., 0, :], sin at [..., 1, :]
        trg_arr = np.ascontiguousarray(
            np.stack([cs_arr, sn_arr], axis=3)).astype(bf)

        in_maps.append({
            "wx": wx_arr, "wy": wy_arr, "enc": enc_arr,
            "trg": trg_arr, "ro": ro_arr,
            "v0b": v0b, "v0t": v0t, "maskd": mask, "ident": ident,
        })
    return in_maps


_NC_CACHE = {}


def get_nc(nlayers: int = L):
    if nlayers not in _NC_CACHE:
        _NC_CACHE[nlayers] = build(nlayers)
    return _NC_CACHE[nlayers]


def kernel(**inputs) -> np.ndarray:
    nc = get_nc()
    in_maps = prep_inputs(inputs)
    res = run_bass_kernel_spmd(nc, in_maps, core_ids=list(range(NCORES)))
    out = res.results[0]["out"].astype(np.float32)
    return out.reshape(1, T, VOCAB)
